# revision 5
# baseline (speedup 1.0000x reference)
"""Trainium2 Bass kernel for the DiffRenderer problem.

Math (per grid cell): probs = softmax(grid_logits[r, c, :]); each cell's
28x14 tile = sum_n probs[n] * font[n]; tiles assembled into a (10752, 10752)
image.

Strategy (8 cores, data-parallel over grid rows — 48 rows per core):
  - Host shards grid_logits by row band, converts to fp16 (halves input
    HBM traffic; logits are N(0,1) so fp16 rounding is ~1e-3 relative on
    probs) and lays each band out as logitsT [69 chars, 36864 cells] so the
    char axis is on SBUF partitions.
  - The whole 5.1MB band preloads into SBUF via 6 SWDGE triggers (gpsimd
    ring) — no per-tile input DMA, so ACT never stalls waiting for loads.
  - Per load tile of 3072 cells: one ACT exp [69, 3072] fp16; then 6 PSUM
    groups of 4 matmuls (fp16, N=394 incl a ones column for the softmax
    denominator) into [128, 2048] (4 banks); one batched DVE reciprocal per
    group; normalize+fp32->fp16 convert fused into the PSUM->SBUF move:
      * 46 groups: ONE DVE tensor_tensor per group (reciprocal broadcast
        along the free axis via stride-0 AP) — amortizes the 120-cycle PSUM
        access latency over 1568 elements
      * 26 groups: 4 ACT copy-with-scale instructions
    split chosen so DVE and ACT land ~90us busy each.
  - DMA out per half tile (1.2MB contiguous, SP ring); last tile splits the
    tail finer to shorten the drain.
  - Device output is the reference's soft_tiles data in a DMA-friendly
    permutation; the host performs the pure reindex to image form — the
    same transpose/reshape the reference itself performs after the math.
"""

import os
from contextlib import ExitStack

import numpy as np

os.environ.setdefault("MYCRO_LOCAL_CACHE", "1")

import concourse.bass as bass  # noqa: F401
import concourse.tile as tile
from concourse import bacc, mybir
from concourse.bass_utils import run_bass_kernel_spmd


def _install_ntff_hook_shim():
    """The image's antenv lacks axon_hooks, but run_bass_kernel_spmd imports
    it whenever BASS_TRACE is set. Provide the module and register the
    ctypes-based NTFF profile hook (degrades to no tracing if unavailable)."""
    import sys
    import types

    if "antenv.axon_hooks" in sys.modules:
        return
    try:
        import antenv
    except ImportError:
        return
    mod = types.ModuleType("antenv.axon_hooks")
    mod._hook = None
    mod.set_axon_ntff_profile_hook = lambda h: setattr(mod, "_hook", h)
    mod.get_axon_ntff_profile_hook = lambda: mod._hook
    sys.modules["antenv.axon_hooks"] = mod
    antenv.axon_hooks = mod
    try:
        from trn_agent_boot.trn_boot import _ntff_profile_via_ctypes

        hook = _ntff_profile_via_ctypes("/opt/axon/libaxon_pjrt.so")
        if hook is not None:
            mod.set_axon_ntff_profile_hook(hook)
    except Exception:
        pass


_install_ntff_hook_shim()

# Problem constants (hardcoded per harness contract)
ROWS, COLS, N_CHARS = 384, 768, 69
CH, CW = 28, 14
HW = CH * CW  # 392
NPAD = HW + 2  # col 392 = ones (softmax denom); 393 = pad (even free size)
N_CORES = 8
ROWS_PER_CORE = ROWS // N_CORES  # 48
CELLS = ROWS_PER_CORE * COLS  # 36864 cells per core
P = 128  # matmul output partitions (cells per chunk)
J = 24  # chunks per load tile
CT = P * J  # 3072 cells per load tile
T = CELLS // CT  # 12 load tiles per core
G = J // 4  # 6 PSUM groups of 4 chunks per load tile
GW = 512  # psum cols per chunk slot (one 2KB bank)
F32 = mybir.dt.float32
F16 = mybir.dt.float16

# Stash of the last run's BassKernelResults (test.py reads exec_time_ns).
LAST_RESULTS = None
_CACHED_NC = None


def _act_groups(t):
    """Groups converted on ACT (scalar) — the rest go to DVE as one fused
    tensor_tensor each. 26 ACT / 46 DVE groups total balances the engines
    (ACT also runs the exps). The last tile ends on a DVE group so the
    drain is one short instruction."""
    if t < 10:
        return (1, 4)
    if t == 10:
        return (1, 3, 5)
    return (1, 3, 4)


def _build_bass():
    nc = bacc.Bacc("TRN2", target_bir_lowering=False, debug=False,
                   num_devices=N_CORES)

    logits_h = nc.dram_tensor("logitsT", [N_CHARS, CELLS], F16,
                              kind="ExternalInput")
    fontb_h = nc.dram_tensor("fontb", [N_CHARS, NPAD], F16,
                             kind="ExternalInput")
    # out[t, p, j, :] holds soft_tiles for cell t*3072 + j*128 + p.
    out_h = nc.dram_tensor("out", [CELLS, HW], F16, kind="ExternalOutput")

    with tile.TileContext(nc) as tc, ExitStack() as ctx:
        singles = ctx.enter_context(tc.tile_pool(name="singles", bufs=1))
        expp = ctx.enter_context(tc.tile_pool(name="expp", bufs=3))
        outp = ctx.enter_context(tc.tile_pool(name="outp", bufs=3))
        rcpp = ctx.enter_context(tc.tile_pool(name="rcpp", bufs=8))
        # [128, 2048] = four PSUM banks per group tile; 2 tiles = all 8 banks
        psp = ctx.enter_context(tc.tile_pool(name="psp", bufs=2,
                                             space="PSUM"))

        fontb_sb = singles.tile([N_CHARS, NPAD], F16)
        nc.sync.dma_start(fontb_sb, fontb_h[:])

        # Whole input band resident in SBUF (72KB/partition on 69 rows).
        # Early segments are small so the first exp starts ~2us in; SWDGE
        # (gpsimd ring) spreads 69-partition descriptors across all 16 DMA
        # engines.
        lg_sb = singles.tile([N_CHARS, CELLS], F16)
        HCT = CT // 2
        seg_bounds = [0, HCT, 2 * HCT, 6 * HCT, 12 * HCT, 18 * HCT, 24 * HCT]
        lg_v = logits_h[:]
        for a, b in zip(seg_bounds[:-1], seg_bounds[1:]):
            nc.gpsimd.dma_start(lg_sb[:, a:b], lg_v[:, a:b])

        out_v = out_h[:].rearrange("(t p j) f -> t p (j f)", p=P, j=J)

        eTs = [None] * T
        eTs[0] = expp.tile([N_CHARS, CT], F16, name="eT")
        # tile 0's exp in halves so the first matmul group starts sooner
        for h in range(2):
            sl = slice(h * HCT, (h + 1) * HCT)
            nc.scalar.activation(eTs[0][:, sl], lg_sb[:, sl],
                                 mybir.ActivationFunctionType.Exp)

        for t in range(T):
            if t + 1 < T:
                eTs[t + 1] = expp.tile([N_CHARS, CT], F16, name="eT")
                nc.scalar.activation(
                    eTs[t + 1][:], lg_sb[:, (t + 1) * CT:(t + 2) * CT],
                    mybir.ActivationFunctionType.Exp)
            eT = eTs[t]
            acts = _act_groups(t)
            out_tile = outp.tile([P, J * HW], F16)
            for g in range(G):
                psm = psp.tile([P, 4 * GW], F32)
                for k in range(4):
                    j = 4 * g + k
                    nc.tensor.matmul(psm[:, k * GW:k * GW + NPAD],
                                     eT[:, j * P:(j + 1) * P], fontb_sb[:],
                                     start=True, stop=True)
                psm_v = psm[:].rearrange("p (c f) -> p c f", c=4)
                rc = rcpp.tile([P, 4], F32)
                rc_v = rc[:].rearrange("p (c f) -> p c f", c=4)
                nc.vector.reciprocal(rc_v, psm_v[:, :, HW:HW + 1])
                if g in acts:
                    for k in range(4):
                        j = 4 * g + k
                        nc.scalar.mul(out_tile[:, j * HW:(j + 1) * HW],
                                      psm[:, k * GW:k * GW + HW],
                                      rc[:, k:k + 1])
                else:
                    dst = out_tile[:, 4 * g * HW:(4 * g + 4) * HW]
                    nc.vector.tensor_tensor(
                        dst.rearrange("p (c f) -> p c f", c=4),
                        psm_v[:, :, 0:HW],
                        rc_v.to_broadcast((P, 4, HW)),
                        mybir.AluOpType.mult)
                if g == 2:
                    nc.sync.dma_start(out_v[t][:, :J * HW // 2],
                                      out_tile[:, :J * HW // 2])
                elif g == 4 and t == T - 1:
                    nc.sync.dma_start(out_v[t][:, 12 * HW:20 * HW],
                                      out_tile[:, 12 * HW:20 * HW])
            if t == T - 1:
                nc.sync.dma_start(out_v[t][:, 20 * HW:],
                                  out_tile[:, 20 * HW:])
            else:
                nc.sync.dma_start(out_v[t][:, J * HW // 2:],
                                  out_tile[:, J * HW // 2:])

    nc.compile()
    return nc


def kernel(grid_logits: np.ndarray, font: np.ndarray) -> np.ndarray:
    global LAST_RESULTS, _CACHED_NC
    grid_logits = np.asarray(grid_logits, dtype=np.float32)
    font = np.asarray(font, dtype=np.float32)
    assert grid_logits.shape == (ROWS, COLS, N_CHARS)
    assert font.shape == (N_CHARS, CH, CW)

    fontb = np.zeros((N_CHARS, NPAD), dtype=np.float32)
    fontb[:, :HW] = font.reshape(N_CHARS, HW)
    fontb[:, HW] = 1.0
    fontb = fontb.astype(np.float16)

    # (69, 384, 768) with chars leading: one big transpose, then per-core
    # contiguous band slices, downcast to fp16 for half the load traffic
    glT = grid_logits.transpose(2, 0, 1)

    in_maps = []
    for k in range(N_CORES):
        band = glT[:, k * ROWS_PER_CORE:(k + 1) * ROWS_PER_CORE, :]
        bandc = np.ascontiguousarray(band, dtype=np.float16)
        in_maps.append({
            "logitsT": bandc.reshape(N_CHARS, CELLS),
            "fontb": fontb,
        })

    if _CACHED_NC is None:
        _CACHED_NC = _build_bass()

    res = run_bass_kernel_spmd(_CACHED_NC, in_maps,
                               core_ids=list(range(N_CORES)))
    LAST_RESULTS = res

    img = np.empty((ROWS * CH, COLS * CW), dtype=np.float32)
    band_h = ROWS_PER_CORE * CH  # 1344
    for k in range(N_CORES):
        arr = res.results[k]["out"].reshape(T, P, J, CH, CW)
        # [t, p, j] holds cell t*3072 + j*128 + p -> reorder to cell-major
        cells = arr.transpose(0, 2, 1, 3, 4).reshape(
            ROWS_PER_CORE, COLS, CH, CW)
        img[k * band_h:(k + 1) * band_h] = (
            cells.transpose(0, 2, 1, 3).reshape(band_h, COLS * CW))
    return img[None, None]


# revision 7
# speedup vs baseline: 1.4177x; 1.4177x over previous
"""Trainium2 Bass kernel for the DiffRenderer problem.

Math (per grid cell): probs = softmax(grid_logits[r, c, :]); each cell's
28x14 tile = sum_n probs[n] * font[n]; tiles assembled into a (10752, 10752)
image.

Strategy (8 cores, data-parallel over grid rows — 48 rows per core):
  - Host shards grid_logits by row band, converts to fp16 (halves input
    HBM traffic; logits are N(0,1) so fp16 rounding is ~1e-3 relative on
    probs) and lays each band out as logitsT [69 chars, 36864 cells] so the
    char axis is on SBUF partitions.
  - The whole 5.1MB band preloads into SBUF via 6 SWDGE triggers (gpsimd
    ring) — no per-tile input DMA, so ACT never stalls waiting for loads.
  - Per load tile of 3072 cells: one ACT exp [69, 3072] fp16; then 6 PSUM
    groups of 4 matmuls (fp16, N=394 incl a ones column for the softmax
    denominator) into [128, 2048] (4 banks); one batched DVE reciprocal per
    group; normalize+fp32->fp16 convert fused into the PSUM->SBUF move:
      * 46 groups: ONE DVE tensor_tensor per group (reciprocal broadcast
        along the free axis via stride-0 AP) — amortizes the 120-cycle PSUM
        access latency over 1568 elements
      * 26 groups: 4 ACT copy-with-scale instructions
    split chosen so DVE and ACT land ~90us busy each.
  - DMA out per half tile (1.2MB contiguous, SP ring); last tile splits the
    tail finer to shorten the drain.
  - Device output is the reference's soft_tiles data in a DMA-friendly
    permutation; the host performs the pure reindex to image form — the
    same transpose/reshape the reference itself performs after the math.
"""

import os
from contextlib import ExitStack

import numpy as np

os.environ.setdefault("MYCRO_LOCAL_CACHE", "1")

import concourse.bass as bass  # noqa: F401
import concourse.tile as tile
from concourse import bacc, mybir
from concourse.bass_utils import run_bass_kernel_spmd


def _install_ntff_hook_shim():
    """The image's antenv lacks axon_hooks, but run_bass_kernel_spmd imports
    it whenever BASS_TRACE is set. Provide the module and register the
    ctypes-based NTFF profile hook (degrades to no tracing if unavailable)."""
    import sys
    import types

    if "antenv.axon_hooks" in sys.modules:
        return
    try:
        import antenv
    except ImportError:
        return
    mod = types.ModuleType("antenv.axon_hooks")
    mod._hook = None
    mod.set_axon_ntff_profile_hook = lambda h: setattr(mod, "_hook", h)
    mod.get_axon_ntff_profile_hook = lambda: mod._hook
    sys.modules["antenv.axon_hooks"] = mod
    antenv.axon_hooks = mod
    try:
        from trn_agent_boot.trn_boot import _ntff_profile_via_ctypes

        hook = _ntff_profile_via_ctypes("/opt/axon/libaxon_pjrt.so")
        if hook is not None:
            mod.set_axon_ntff_profile_hook(hook)
    except Exception:
        pass


_install_ntff_hook_shim()

# Problem constants (hardcoded per harness contract)
ROWS, COLS, N_CHARS = 384, 768, 69
CH, CW = 28, 14
HW = CH * CW  # 392
NPAD = HW + 2  # col 392 = ones (softmax denom); 393 = pad (even free size)
N_CORES = 8
ROWS_PER_CORE = ROWS // N_CORES  # 48
CELLS = ROWS_PER_CORE * COLS  # 36864 cells per core
P = 128  # matmul output partitions (cells per chunk)
J = 24  # chunks per load tile
CT = P * J  # 3072 cells per load tile
T = CELLS // CT  # 12 load tiles per core
G = J // 4  # 6 PSUM groups of 4 chunks per load tile
GW = 512  # psum cols per chunk slot (one 2KB bank)
F32 = mybir.dt.float32
F16 = mybir.dt.float16

# Stash of the last run's BassKernelResults (test.py reads exec_time_ns).
LAST_RESULTS = None
_CACHED_NC = None


def _act_pairs(t):
    """Chunk-pairs converted on ACT (scalar) — the rest go to DVE as one
    fused tensor_tensor each. 51 ACT / 93 DVE pairs total balances the
    engines (ACT also runs the exps). The last tile ends on a DVE pair so
    the drain is one short instruction."""
    if t < 9:
        return (1, 4, 7, 10)
    return (1, 2, 4, 7, 10)


def _build_bass():
    nc = bacc.Bacc("TRN2", target_bir_lowering=False, debug=False,
                   num_devices=N_CORES)

    logits_h = nc.dram_tensor("logitsT", [N_CHARS, CELLS], F16,
                              kind="ExternalInput")
    fontb_h = nc.dram_tensor("fontb", [N_CHARS, NPAD], F16,
                             kind="ExternalInput")
    # out[t, p, j, :] holds soft_tiles for cell t*3072 + j*128 + p.
    out_h = nc.dram_tensor("out", [CELLS, HW], F16, kind="ExternalOutput")

    with tile.TileContext(nc) as tc, ExitStack() as ctx:
        singles = ctx.enter_context(tc.tile_pool(name="singles", bufs=1))
        lgp = ctx.enter_context(tc.tile_pool(name="lgp", bufs=T))
        expp = ctx.enter_context(tc.tile_pool(name="expp", bufs=3))
        outp = ctx.enter_context(tc.tile_pool(name="outp", bufs=3))
        rcpp = ctx.enter_context(tc.tile_pool(name="rcpp", bufs=8))
        # [128, 1024] = two PSUM banks per pair tile; 4 tiles = all 8 banks
        psp = ctx.enter_context(tc.tile_pool(name="psp", bufs=4,
                                             space="PSUM"))

        fontb_sb = singles.tile([N_CHARS, NPAD], F16)
        nc.sync.dma_start(fontb_sb, fontb_h[:])

        # All 12 input tiles issued up front on the SWDGE (gpsimd) ring —
        # it spreads 69-partition descriptors across all 16 DMA engines,
        # and per-tile tiles keep the exp deps fine-grained. Tile 0 loads
        # in halves so the first exp starts as early as possible.
        HCT = CT // 2
        lg_v = logits_h[:]
        lgs = []
        for t in range(T):
            lg_t = lgp.tile([N_CHARS, CT], F16, name="lg")
            lgs.append(lg_t)
            if t == 0:
                for h in range(2):
                    sl = slice(h * HCT, (h + 1) * HCT)
                    nc.gpsimd.dma_start(lg_t[:, sl], lg_v[:, sl])
            else:
                nc.gpsimd.dma_start(lg_t[:],
                                    lg_v[:, t * CT:(t + 1) * CT])

        out_v = out_h[:].rearrange("(t p j) f -> t p (j f)", p=P, j=J)

        eTs = [None] * T
        eTs[0] = expp.tile([N_CHARS, CT], F16, name="eT")
        # tile 0's exp in halves so the first matmul pair starts sooner
        for h in range(2):
            sl = slice(h * HCT, (h + 1) * HCT)
            nc.scalar.activation(eTs[0][:, sl], lgs[0][:, sl],
                                 mybir.ActivationFunctionType.Exp)

        NPAIR = J // 2  # 12 chunk-pairs per load tile
        for t in range(T):
            if t + 1 < T:
                eTs[t + 1] = expp.tile([N_CHARS, CT], F16, name="eT")
            eT = eTs[t]
            acts = _act_pairs(t)
            out_tile = outp.tile([P, J * HW], F16)
            for q in range(NPAIR):
                psm = psp.tile([P, 2 * GW], F32)
                for k in range(2):
                    j = 2 * q + k
                    nc.tensor.matmul(psm[:, k * GW:k * GW + NPAD],
                                     eT[:, j * P:(j + 1) * P], fontb_sb[:],
                                     start=True, stop=True)
                psm_v = psm[:].rearrange("p (c f) -> p c f", c=2)
                rc = rcpp.tile([P, 2], F32)
                rc_v = rc[:].rearrange("p (c f) -> p c f", c=2)
                nc.vector.reciprocal(rc_v, psm_v[:, :, HW:HW + 1])
                if q in acts:
                    for k in range(2):
                        j = 2 * q + k
                        nc.scalar.mul(out_tile[:, j * HW:(j + 1) * HW],
                                      psm[:, k * GW:k * GW + HW],
                                      rc[:, k:k + 1])
                else:
                    dst = out_tile[:, 2 * q * HW:(2 * q + 2) * HW]
                    nc.vector.tensor_tensor(
                        dst.rearrange("p (c f) -> p c f", c=2),
                        psm_v[:, :, 0:HW],
                        rc_v.to_broadcast((P, 2, HW)),
                        mybir.AluOpType.mult)
                # exp for the next tile, interleaved between converts so it
                # never head-of-line blocks ACT's PSUM-freeing copies long
                if t + 1 < T and q in (2, 7):
                    h = 0 if q == 2 else 1
                    sl = slice(h * HCT, (h + 1) * HCT)
                    nc.scalar.activation(
                        eTs[t + 1][:, sl], lgs[t + 1][:, sl],
                        mybir.ActivationFunctionType.Exp)
                if q == 5:
                    nc.sync.dma_start(out_v[t][:, :J * HW // 2],
                                      out_tile[:, :J * HW // 2])
                elif q == 9 and t == T - 1:
                    nc.sync.dma_start(out_v[t][:, 12 * HW:20 * HW],
                                      out_tile[:, 12 * HW:20 * HW])
            if t == T - 1:
                nc.sync.dma_start(out_v[t][:, 20 * HW:],
                                  out_tile[:, 20 * HW:])
            else:
                nc.sync.dma_start(out_v[t][:, J * HW // 2:],
                                  out_tile[:, J * HW // 2:])

    nc.compile()
    return nc


def kernel(grid_logits: np.ndarray, font: np.ndarray) -> np.ndarray:
    global LAST_RESULTS, _CACHED_NC
    grid_logits = np.asarray(grid_logits, dtype=np.float32)
    font = np.asarray(font, dtype=np.float32)
    assert grid_logits.shape == (ROWS, COLS, N_CHARS)
    assert font.shape == (N_CHARS, CH, CW)

    fontb = np.zeros((N_CHARS, NPAD), dtype=np.float32)
    fontb[:, :HW] = font.reshape(N_CHARS, HW)
    fontb[:, HW] = 1.0
    fontb = fontb.astype(np.float16)

    # (69, 384, 768) with chars leading: one big transpose, then per-core
    # contiguous band slices, downcast to fp16 for half the load traffic
    glT = grid_logits.transpose(2, 0, 1)

    in_maps = []
    for k in range(N_CORES):
        band = glT[:, k * ROWS_PER_CORE:(k + 1) * ROWS_PER_CORE, :]
        bandc = np.ascontiguousarray(band, dtype=np.float16)
        in_maps.append({
            "logitsT": bandc.reshape(N_CHARS, CELLS),
            "fontb": fontb,
        })

    if _CACHED_NC is None:
        _CACHED_NC = _build_bass()

    res = run_bass_kernel_spmd(_CACHED_NC, in_maps,
                               core_ids=list(range(N_CORES)))
    LAST_RESULTS = res

    img = np.empty((ROWS * CH, COLS * CW), dtype=np.float32)
    band_h = ROWS_PER_CORE * CH  # 1344
    for k in range(N_CORES):
        arr = res.results[k]["out"].reshape(T, P, J, CH, CW)
        # [t, p, j] holds cell t*3072 + j*128 + p -> reorder to cell-major
        cells = arr.transpose(0, 2, 1, 3, 4).reshape(
            ROWS_PER_CORE, COLS, CH, CW)
        img[k * band_h:(k + 1) * band_h] = (
            cells.transpose(0, 2, 1, 3).reshape(band_h, COLS * CW))
    return img[None, None]


# revision 10
# speedup vs baseline: 1.4450x; 1.0193x over previous
"""Trainium2 Bass kernel for the DiffRenderer problem.

Math (per grid cell): probs = softmax(grid_logits[r, c, :]); each cell's
28x14 tile = sum_n probs[n] * font[n]; tiles assembled into a (10752, 10752)
image.

Strategy (8 cores, data-parallel over grid rows — 48 rows per core):
  - Host shards grid_logits by row band, converts to fp16 (halves input
    HBM traffic; logits are N(0,1) so fp16 rounding is ~1e-3 relative on
    probs) and lays each band out as logitsT [69 chars, 36864 cells] so the
    char axis is on SBUF partitions.
  - The whole 5.1MB band preloads into SBUF via 6 SWDGE triggers (gpsimd
    ring) — no per-tile input DMA, so ACT never stalls waiting for loads.
  - Per load tile of 3072 cells: one ACT exp [69, 3072] fp16; then 6 PSUM
    groups of 4 matmuls (fp16, N=394 incl a ones column for the softmax
    denominator) into [128, 2048] (4 banks); one batched DVE reciprocal per
    group; normalize+fp32->fp16 convert fused into the PSUM->SBUF move:
      * 46 groups: ONE DVE tensor_tensor per group (reciprocal broadcast
        along the free axis via stride-0 AP) — amortizes the 120-cycle PSUM
        access latency over 1568 elements
      * 26 groups: 4 ACT copy-with-scale instructions
    split chosen so DVE and ACT land ~90us busy each.
  - DMA out per half tile (1.2MB contiguous, SP ring); last tile splits the
    tail finer to shorten the drain.
  - Device output is the reference's soft_tiles data in a DMA-friendly
    permutation; the host performs the pure reindex to image form — the
    same transpose/reshape the reference itself performs after the math.
"""

import os
from contextlib import ExitStack

import numpy as np

os.environ.setdefault("MYCRO_LOCAL_CACHE", "1")

import concourse.bass as bass  # noqa: F401
import concourse.tile as tile
from concourse import bacc, mybir
from concourse.bass_utils import run_bass_kernel_spmd


def _install_ntff_hook_shim():
    """The image's antenv lacks axon_hooks, but run_bass_kernel_spmd imports
    it whenever BASS_TRACE is set. Provide the module and register the
    ctypes-based NTFF profile hook (degrades to no tracing if unavailable)."""
    import sys
    import types

    if "antenv.axon_hooks" in sys.modules:
        return
    try:
        import antenv
    except ImportError:
        return
    mod = types.ModuleType("antenv.axon_hooks")
    mod._hook = None
    mod.set_axon_ntff_profile_hook = lambda h: setattr(mod, "_hook", h)
    mod.get_axon_ntff_profile_hook = lambda: mod._hook
    sys.modules["antenv.axon_hooks"] = mod
    antenv.axon_hooks = mod
    try:
        from trn_agent_boot.trn_boot import _ntff_profile_via_ctypes

        hook = _ntff_profile_via_ctypes("/opt/axon/libaxon_pjrt.so")
        if hook is not None:
            mod.set_axon_ntff_profile_hook(hook)
    except Exception:
        pass


_install_ntff_hook_shim()

# Problem constants (hardcoded per harness contract)
ROWS, COLS, N_CHARS = 384, 768, 69
CH, CW = 28, 14
HW = CH * CW  # 392
NPAD = HW + 2  # col 392 = ones (softmax denom); 393 = pad (even free size)
N_CORES = 8
ROWS_PER_CORE = ROWS // N_CORES  # 48
CELLS = ROWS_PER_CORE * COLS  # 36864 cells per core
P = 128  # matmul output partitions (cells per chunk)
J = 24  # chunks per load tile
CT = P * J  # 3072 cells per load tile
T = CELLS // CT  # 12 load tiles per core
G = J // 4  # 6 PSUM groups of 4 chunks per load tile
GW = 512  # psum cols per chunk slot (one 2KB bank)
F32 = mybir.dt.float32
F16 = mybir.dt.float16

# Stash of the last run's BassKernelResults (test.py reads exec_time_ns).
LAST_RESULTS = None
_CACHED_NC = None


def _act_pairs(t):
    """Chunk-pairs converted on ACT (scalar) — the rest go to DVE as one
    fused tensor_tensor each. 51 ACT / 93 DVE pairs total balances the
    engines (ACT also runs the exps). The last tile ends on a DVE pair so
    the drain is one short instruction."""
    if t < 9:
        return (1, 4, 7, 10)
    return (1, 3, 5, 7, 9)


def _build_bass():
    nc = bacc.Bacc("TRN2", target_bir_lowering=False, debug=False,
                   num_devices=N_CORES)

    logits_h = nc.dram_tensor("logitsT", [N_CHARS, CELLS], F16,
                              kind="ExternalInput")
    fontb_h = nc.dram_tensor("fontb", [N_CHARS, NPAD], F16,
                             kind="ExternalInput")
    # out[t, p, j, :] holds soft_tiles for cell t*3072 + j*128 + p.
    out_h = nc.dram_tensor("out", [CELLS, HW], F16, kind="ExternalOutput")

    with tile.TileContext(nc) as tc, ExitStack() as ctx:
        singles = ctx.enter_context(tc.tile_pool(name="singles", bufs=1))
        lgp = ctx.enter_context(tc.tile_pool(name="lgp", bufs=T))
        expp = ctx.enter_context(tc.tile_pool(name="expp", bufs=3))
        outp = ctx.enter_context(tc.tile_pool(name="outp", bufs=3))
        rcpp = ctx.enter_context(tc.tile_pool(name="rcpp", bufs=8))
        # [128, 1024] = two PSUM banks per pair tile; 4 tiles = all 8 banks
        psp = ctx.enter_context(tc.tile_pool(name="psp", bufs=4,
                                             space="PSUM"))

        fontb_sb = singles.tile([N_CHARS, NPAD], F16)
        nc.sync.dma_start(fontb_sb, fontb_h[:])

        # Tile 0's first two quarters ride the sync HWDGE ring right behind
        # the font so the first exp runs as soon as the preamble ends; the
        # rest arrives via SWDGE (gpsimd ring), which spreads 69-partition
        # descriptors across all 16 DMA engines. Per-tile tiles keep the
        # exp deps fine-grained.
        QCT = CT // 4
        HCT = CT // 2
        lg_v = logits_h[:]
        lgs = [lgp.tile([N_CHARS, CT], F16, name="lg") for _ in range(T)]
        nc.sync.dma_start(lgs[0][:, :QCT], lg_v[:, :QCT])
        nc.sync.dma_start(lgs[0][:, QCT:HCT], lg_v[:, QCT:HCT])
        nc.gpsimd.dma_start(lgs[0][:, HCT:], lg_v[:, HCT:CT])
        for t in range(1, T):
            nc.gpsimd.dma_start(lgs[t][:], lg_v[:, t * CT:(t + 1) * CT])

        out_v = out_h[:].rearrange("(t p j) f -> t p (j f)", p=P, j=J)

        eTs = [None] * T
        eTs[0] = expp.tile([N_CHARS, CT], F16, name="eT")
        # tile 0's exp in quarters so the first matmul pair starts sooner
        for h in range(4):
            sl = slice(h * QCT, (h + 1) * QCT)
            nc.scalar.activation(eTs[0][:, sl], lgs[0][:, sl],
                                 mybir.ActivationFunctionType.Exp)

        NPAIR = J // 2  # 12 chunk-pairs per load tile
        for t in range(T):
            if t + 1 < T:
                eTs[t + 1] = expp.tile([N_CHARS, CT], F16, name="eT")
            eT = eTs[t]
            acts = _act_pairs(t)
            out_tile = outp.tile([P, J * HW], F16)
            for q in range(NPAIR):
                psm = psp.tile([P, 2 * GW], F32)
                for k in range(2):
                    j = 2 * q + k
                    nc.tensor.matmul(psm[:, k * GW:k * GW + NPAD],
                                     eT[:, j * P:(j + 1) * P], fontb_sb[:],
                                     start=True, stop=True)
                psm_v = psm[:].rearrange("p (c f) -> p c f", c=2)
                rc = rcpp.tile([P, 2], F32)
                rc_v = rc[:].rearrange("p (c f) -> p c f", c=2)
                nc.vector.reciprocal(rc_v, psm_v[:, :, HW:HW + 1])
                if q in acts:
                    for k in range(2):
                        j = 2 * q + k
                        nc.scalar.mul(out_tile[:, j * HW:(j + 1) * HW],
                                      psm[:, k * GW:k * GW + HW],
                                      rc[:, k:k + 1])
                else:
                    dst = out_tile[:, 2 * q * HW:(2 * q + 2) * HW]
                    nc.vector.tensor_tensor(
                        dst.rearrange("p (c f) -> p c f", c=2),
                        psm_v[:, :, 0:HW],
                        rc_v.to_broadcast((P, 2, HW)),
                        mybir.AluOpType.mult)
                # exp for the next tile, in quarters interleaved between
                # converts so it never head-of-line blocks ACT's
                # PSUM-freeing copies for long
                if t + 1 < T and q in (2, 5, 8, 11):
                    h = (2, 5, 8, 11).index(q)
                    sl = slice(h * QCT, (h + 1) * QCT)
                    nc.scalar.activation(
                        eTs[t + 1][:, sl], lgs[t + 1][:, sl],
                        mybir.ActivationFunctionType.Exp)
                if q == 5:
                    nc.sync.dma_start(out_v[t][:, :J * HW // 2],
                                      out_tile[:, :J * HW // 2])
                elif q == 9 and t == T - 1:
                    nc.sync.dma_start(out_v[t][:, 12 * HW:20 * HW],
                                      out_tile[:, 12 * HW:20 * HW])
            if t == T - 1:
                nc.sync.dma_start(out_v[t][:, 20 * HW:],
                                  out_tile[:, 20 * HW:])
            else:
                nc.sync.dma_start(out_v[t][:, J * HW // 2:],
                                  out_tile[:, J * HW // 2:])

    nc.compile()
    return nc


def kernel(grid_logits: np.ndarray, font: np.ndarray) -> np.ndarray:
    global LAST_RESULTS, _CACHED_NC
    grid_logits = np.asarray(grid_logits, dtype=np.float32)
    font = np.asarray(font, dtype=np.float32)
    assert grid_logits.shape == (ROWS, COLS, N_CHARS)
    assert font.shape == (N_CHARS, CH, CW)

    fontb = np.zeros((N_CHARS, NPAD), dtype=np.float32)
    fontb[:, :HW] = font.reshape(N_CHARS, HW)
    fontb[:, HW] = 1.0
    fontb = fontb.astype(np.float16)

    # (69, 384, 768) with chars leading: one big transpose, then per-core
    # contiguous band slices, downcast to fp16 for half the load traffic
    glT = grid_logits.transpose(2, 0, 1)

    in_maps = []
    for k in range(N_CORES):
        band = glT[:, k * ROWS_PER_CORE:(k + 1) * ROWS_PER_CORE, :]
        bandc = np.ascontiguousarray(band, dtype=np.float16)
        in_maps.append({
            "logitsT": bandc.reshape(N_CHARS, CELLS),
            "fontb": fontb,
        })

    if _CACHED_NC is None:
        _CACHED_NC = _build_bass()

    res = run_bass_kernel_spmd(_CACHED_NC, in_maps,
                               core_ids=list(range(N_CORES)))
    LAST_RESULTS = res

    img = np.empty((ROWS * CH, COLS * CW), dtype=np.float32)
    band_h = ROWS_PER_CORE * CH  # 1344
    for k in range(N_CORES):
        arr = res.results[k]["out"].reshape(T, P, J, CH, CW)
        # [t, p, j] holds cell t*3072 + j*128 + p -> reorder to cell-major
        cells = arr.transpose(0, 2, 1, 3, 4).reshape(
            ROWS_PER_CORE, COLS, CH, CW)
        img[k * band_h:(k + 1) * band_h] = (
            cells.transpose(0, 2, 1, 3).reshape(band_h, COLS * CW))
    return img[None, None]
